# revision 44
# baseline (speedup 1.0000x reference)
"""Trainium2 Bass kernel: 2-layer GCN (PyG-style GCNConv x2) on 8 NeuronCores.

Strategy:
  - Nodes sharded contiguously across 8 cores (12500 rows each).
  - Per layer: dense h' = (x @ W) * dinv[row] computed on the owning core,
    AllGather h' to every core (51MB replica), then per-core sparse
    aggregation over its in-edges:
      gather h'[src] rows via dma_gather (int16 idx -> 4 src blocks of 25000),
      scatter-add via one-hot matmul into PSUM per 128-dst tile,
      bias added as rank-1 matmul outer(sqrt(deg), b),
      eviction scaled by dinv[dst] on the scalar engine.
  - The per-edge norm dinv[src]*dinv[dst] is folded into the two node-level
    scalings, so no per-edge multiply exists anywhere.
"""

import os
import sys

for _p in ("/opt/trn_rl_repo",):
    if _p not in sys.path:
        sys.path.append(_p)

import numpy as np
import ml_dtypes

import concourse.bacc as bacc
import concourse.bass as bass
import concourse.mybir as mybir
import concourse.tile as tile
from concourse.bass_utils import run_bass_kernel_spmd

F32 = mybir.dt.float32
BF16 = mybir.dt.bfloat16
F16 = mybir.dt.float16
I16 = mybir.dt.int16
AF = mybir.ActivationFunctionType
ALU = mybir.AluOpType
NPBF16 = ml_dtypes.bfloat16
NQ = 4  # SWDGE queues (ring-drain parallelism for dma_gather)

N_NODES = 100000
D = 128
NCORES = 8
TILE = 128


def _ceil_div(a, b):
    return (a + b - 1) // b


class Plan:
    """Core-uniform structure tables derived from the edge index.

    Chunks are laid out per (tile-group, src-block) cell: within a cell a
    core's edges (sorted by dst tile, then src) fill C_gb*128 slots where
    C_gb = max over cores of ceil(count/128). A chunk may span a tile
    boundary; doff holds the GROUP-relative dst offset (0..G*128-1), and
    the per-tile one-hot is built with a -128*t_rel shift so out-of-tile
    slots contribute zero. The (tile -> chunk range) schedule is the union
    over cores, so one SPMD program fits all cores.
    """

    def __init__(self, n_nodes, edge_index, group_tiles=4):
        self.n = n_nodes
        self.ns = n_nodes // NCORES            # nodes per core
        self.nt = _ceil_div(self.ns, TILE)     # dst tiles per core
        self.last_w = self.ns - (self.nt - 1) * TILE
        self.G = group_tiles

        # deg includes self-loops (as the reference adds them), but the
        # self-loop edges themselves are NOT gathered: their contribution
        # h'[dst] is added locally from the dense-phase output via one
        # identity matmul per tile.
        src = np.asarray(edge_index[0])
        dst = np.asarray(edge_index[1])
        deg = (np.bincount(dst, minlength=n_nodes) + 1).astype(np.float32)
        self.dinv = deg ** -0.5
        self.sdeg = np.sqrt(deg)

        # tile groups
        self.groups = [list(range(g0, min(g0 + self.G, self.nt)))
                       for g0 in range(0, self.nt, self.G)]
        ng = len(self.groups)

        # The AllGather is split into 4 tile-aligned chunks, one per gather
        # src-block, so each block's gathers unblock as its chunk lands.
        # Chunk outputs are core-major, so the replica hf holds row i of the
        # original numbering at remapped position frow(i); all gather indices
        # are built against that layout.
        tchunk = [25, 25, 24, self.nt - 74]     # tiles per chunk
        szs, base = [], []
        o = 0
        for k, tc_ in enumerate(tchunk):
            base.append(o)
            if k < 3:
                szs.append(tc_ * TILE)
                o += tc_ * TILE
            else:
                szs.append(self.ns - o)
        self.ag_szs, self.ag_base = szs, base   # per-core chunk row ranges
        self.blo = []
        self.bhi = []
        o = 0
        for s in szs:
            self.blo.append(o)
            self.bhi.append(o + NCORES * s)
            o += NCORES * s
        assert o == n_nodes
        assert max(h - l for l, h in zip(self.blo, self.bhi)) < 32768
        self.nblk = len(self.blo)
        nodes = np.arange(n_nodes, dtype=np.int64)
        r = nodes % self.ns
        j = np.searchsorted(np.asarray(base), r, side="right") - 1
        szs_a = np.asarray(szs)[j]
        self.frow = (np.asarray(self.blo)[j] + (nodes // self.ns) * szs_a
                     + r - np.asarray(base)[j])

        core = dst // self.ns
        dstloc = dst % self.ns
        tloc = dstloc // TILE
        gi = np.minimum(tloc // self.G, ng - 1)
        goff = dstloc - gi * self.G * TILE     # group-relative dst offset
        frow_e = self.frow[src]
        blk = np.searchsorted(np.asarray(self.blo), frow_e, side="right") - 1
        cell = (core * ng + gi) * self.nblk + blk
        # sort: cell, then dst tile, then remapped src row (ascending HBM)
        order = np.lexsort((frow_e, tloc, cell))
        self.src_s = frow_e[order]
        self.goff_s = goff[order]
        ncell = NCORES * ng * self.nblk
        cnt3 = np.bincount(cell, minlength=ncell).reshape(
            NCORES, ng, self.nblk)
        self.cnt3 = cnt3
        self.seg_off = np.zeros(ncell + 1, dtype=np.int64)
        np.cumsum(cnt3.reshape(-1), out=self.seg_off[1:])

        # per (core, g, blk, t_rel) counts -> union (tile -> chunk range)
        cell4 = cell * self.G + (tloc - gi * self.G)
        cnt4 = np.bincount(cell4, minlength=ncell * self.G).reshape(
            NCORES, ng, self.nblk, self.G)
        pre4 = np.cumsum(cnt4, axis=3) - cnt4   # exclusive prefix within cell

        # chunks per (g, b): cross-core max
        self.gb_C = _ceil_div(cnt3, TILE).max(axis=0)  # [ng, nblk]

        # schedule[g][b] -> list of (t_rel, kmin, nk)
        self.sched = []
        for g in range(ng):
            row = []
            for b in range(self.nblk):
                ent = []
                for tr in range(len(self.groups[g])):
                    c4 = cnt4[:, g, b, tr]
                    if not c4.any():
                        continue
                    p4 = pre4[:, g, b, tr]
                    alive = c4 > 0
                    kmin = int((p4[alive] // TILE).min())
                    kmax = int(((p4[alive] + c4[alive] - 1) // TILE).max())
                    ent.append((tr, kmin, kmax - kmin + 1))
                row.append(ent)
            self.sched.append(row)

        # column offsets in the concatenated idx / dstoff DRAM buffers
        self.idx_col = []     # [g][b] -> start col in idx buffer (int16 wrap)
        self.ch_col = []      # [g] -> start chunk col in dstoff buffer
        ic = 0
        cc = 0
        for g in range(ng):
            self.ch_col.append(cc)
            row = []
            for b in range(self.nblk):
                row.append(ic)
                ic += int(self.gb_C[g, b]) * (TILE // 16)
                cc += int(self.gb_C[g, b])
            self.idx_col.append(row)
        self.idx_cols = ic
        self.ch_cols = cc

    def core_inputs(self, c):
        """Build idx (int16 [128, idx_cols]) and dstoff (fp16 [128, ch_cols])."""
        ng = len(self.groups)
        idx = np.zeros((16, self.idx_cols), dtype=np.int16)
        doff = np.full((128, self.ch_cols), -1.0, dtype=np.float16)
        for g in range(ng):
            ch = self.ch_col[g]
            for b in range(self.nblk):
                icol = self.idx_col[g][b]
                nch = int(self.gb_C[g, b])
                cnt = int(self.cnt3[c, g, b])
                o = self.seg_off[(c * ng + g) * self.nblk + b]
                nslots = nch * TILE
                a = np.zeros(nslots, dtype=np.int16)
                a[:cnt] = (self.src_s[o:o + cnt] - self.blo[b]).astype(np.int16)
                idx[:, icol:icol + nch * 8] = a.reshape(nch * 8, 16).T
                dv = np.full(nslots, -1.0, dtype=np.float16)
                dv[:cnt] = self.goff_s[o:o + cnt].astype(np.float16)
                doff[:, ch:ch + nch] = dv.reshape(nch, 128).T
                ch += nch
        idx_full = np.tile(idx, (8, 1))
        return idx_full, doff


def _build(plan, stage="full"):
    """Build the SPMD bass program (shared by all 8 cores)."""
    n, ns, nt, nblk = plan.n, plan.ns, plan.nt, plan.nblk
    nc = bacc.Bacc("TRN2", target_bir_lowering=False, debug=False,
                   num_devices=NCORES, num_swdge_queues=NQ)

    xT = nc.dram_tensor("xT", [D, ns], BF16, kind="ExternalInput").ap()
    wts = nc.dram_tensor("wts", [D, 2 * D], BF16, kind="ExternalInput").ap()
    consts = nc.dram_tensor("consts", [D, 2 * D], BF16, kind="ExternalInput").ap()
    brow = nc.dram_tensor("brow", [1, 2 * D], BF16, kind="ExternalInput").ap()
    dinv_c = nc.dram_tensor("dinv_c", [D, nt], F32, kind="ExternalInput").ap()
    sdeg_r = nc.dram_tensor("sdeg_r", [1, nt * TILE], BF16, kind="ExternalInput").ap()
    idx_d = nc.dram_tensor("idx", [D, plan.idx_cols], I16, kind="ExternalInput").ap()
    doff_d = nc.dram_tensor("doff", [D, plan.ch_cols], F16, kind="ExternalInput").ap()
    iota16_d = nc.dram_tensor("iota16", [D, D], F16, kind="ExternalInput").ap()
    out_d = nc.dram_tensor("out", [ns, D], F32, kind="ExternalOutput").ap()

    hb = [nc.dram_tensor(f"h{i}b", [ns, D], BF16).ap() for i in range(2)]
    hf = [nc.dram_tensor(f"h{i}f", [n, D], BF16, addr_space="Shared").ap()
          for i in range(2)]

    max_C = max(int(plan.gb_C[g].sum()) for g in range(len(plan.groups)))
    max_icols = max_C * 8
    # one-hot columns per (g, b): sum over scheduled tiles of their k-span
    max_ohC = max(sum(nk for (_, _, nk) in plan.sched[g][b])
                  for g in range(len(plan.groups))
                  for b in range(plan.nblk))

    with tile.TileContext(nc) as tc:
        with (
            tc.tile_pool(name="const", bufs=1) as cpool,
            tc.tile_pool(name="xstream", bufs=3) as xpool,
            tc.tile_pool(name="stage", bufs=3) as spool,
            tc.tile_pool(name="oh", bufs=3) as ohpool,
            tc.tile_pool(name="aux", bufs=2) as auxpool,
            tc.tile_pool(name="ev", bufs=4) as evpool,
            tc.tile_pool(name="acc", bufs=5, space="PSUM") as accpool,
            tc.tile_pool(name="ptr", bufs=1, space="PSUM") as trpool,
            tc.tile_pool(name="pd", bufs=2, space="PSUM") as pdpool,
        ):
            w_sb = cpool.tile([D, 2 * D], BF16, tag="w")
            nc.sync.dma_start(out=w_sb[:], in_=wts[:])
            co_sb = cpool.tile([D, 2 * D], BF16, tag="co")
            nc.sync.dma_start(out=co_sb[:], in_=consts[:])
            br_sb = cpool.tile([1, 2 * D], BF16, tag="br")
            nc.sync.dma_start(out=br_sb[:], in_=brow[:])
            dv_sb = cpool.tile([D, nt], F32, tag="dv")
            nc.sync.dma_start(out=dv_sb[:], in_=dinv_c[:])
            sd_sb = cpool.tile([1, nt * TILE], BF16, tag="sd")
            nc.sync.dma_start(out=sd_sb[:], in_=sdeg_r[:])

            io_sb = cpool.tile([D, D], F16, tag="io16")
            nc.sync.dma_start(out=io_sb[:], in_=iota16_d[:])

            W1 = w_sb[:, 0:D]
            W2 = w_sb[:, D:2 * D]
            iota = io_sb[:, 0:D]
            ident = co_sb[:, D:2 * D]

            # dense outputs h{0,1}' stay resident: the self-loop term is
            # added from here straight into PSUM (no gather round-trip)
            hself = [cpool.tile([TILE, nt * D], BF16, tag=f"hself{i}",
                                name=f"hself{i}")
                     for i in range(2)]

            def tw(t):
                return TILE if t < nt - 1 else plan.last_w

            # ---- layer-1 dense: h0' = (x @ W1) * dinv ----
            XB = 8  # xT tiles per DMA batch
            for t0 in range(0, nt, XB):
                tn = min(XB, nt - t0)
                wlast = tw(t0 + tn - 1)
                cols = (tn - 1) * TILE + wlast
                xt_t = xpool.tile([D, XB * TILE], BF16, tag="xt")
                nc.sync.dma_start(out=xt_t[:, :cols],
                                  in_=xT[:, t0 * TILE:t0 * TILE + cols])
                for t in range(t0, t0 + tn):
                    w = tw(t)
                    pd = pdpool.tile([TILE, D], F32, tag="pd")
                    nc.tensor.matmul(
                        pd[:w, :],
                        lhsT=xt_t[:, (t - t0) * TILE:(t - t0) * TILE + w],
                        rhs=W1, start=True, stop=True)
                    # dinv[row] is folded into xT on the host; plain copy,
                    # alternating engines to halve the eviction chain
                    hs = hself[0][:, t * D:(t + 1) * D]
                    if t % 2 == 0:
                        nc.scalar.activation(hs[:w, :], pd[:w, :], AF.Copy)
                    else:
                        nc.vector.tensor_copy(hs[:w, :], pd[:w, :])
                    nc.sync.dma_start(out=hb[0][t * TILE:t * TILE + w, :],
                                      in_=hs[:w, :])

            if stage == "dense":
                nc.gpsimd.collective_compute(
                    "AllGather", ALU.bypass,
                    replica_groups=[list(range(NCORES))],
                    ins=[hb[0].opt()], outs=[hf[0].opt()])
                for t in range(nt):
                    w = tw(t)
                    ev = evpool.tile([TILE, D], BF16, tag="ev")
                    nc.sync.dma_start(out=ev[:w, :],
                                      in_=hf[0][t * TILE:t * TILE + w, :])
                    evf = evpool.tile([TILE, D], F32, tag="evf")
                    nc.scalar.activation(evf[:w, :], ev[:w, :], AF.Copy)
                    nc.sync.dma_start(out=out_d[t * TILE:t * TILE + w, :],
                                      in_=evf[:w, :])

            # ---- sparse layer (templated over layer index) ----
            max_Cgb = max((int(plan.gb_C[g, b]) for g in range(len(plan.groups))
                           for b in range(nblk)), default=1)
            GMAX = 8  # dma_gather caps at ~1024 idxs (16KB desc ring)

            qcnt = [0]  # global dma_gather round-robin across SWDGE queues

            def sparse_layer(li):
                src_full = hf[li]
                for g, tiles in enumerate(plan.groups):
                    Ctot = int(plan.gb_C[g].sum())
                    if Ctot == 0:
                        continue
                    icols = Ctot * 8
                    idx_sb = auxpool.tile([D, max_icols], I16, tag="idx")
                    nc.sync.dma_start(
                        out=idx_sb[:, :icols],
                        in_=idx_d[:, plan.idx_col[g][0]:plan.idx_col[g][0] + icols])
                    do_sb = auxpool.tile([D, max_C], F16, tag="doff")
                    nc.sync.dma_start(
                        out=do_sb[:, :Ctot],
                        in_=doff_d[:, plan.ch_col[g]:plan.ch_col[g] + Ctot])

                    accs = {}
                    started = set()
                    for t in tiles:
                        accs[t] = accpool.tile([TILE, D], F32, tag="acc", name=f"acc_t{t}")
                        # self-loop term: acc[dst] += h'[dst]
                        w = tw(t)
                        nc.tensor.matmul(
                            accs[t][:w, :], lhsT=ident[:w, :w],
                            rhs=hself[li][:w, t * D:(t + 1) * D],
                            start=True, stop=False)
                        started.add(t)

                    gco = 0  # chunk offset within the group's doff columns
                    for b in range(nblk):
                        Cgb = int(plan.gb_C[g, b])
                        if Cgb == 0:
                            continue
                        sched = plan.sched[g][b]
                        ic0 = plan.idx_col[g][b] - plan.idx_col[g][0]
                        stg = spool.tile([D, max_Cgb, TILE], BF16, tag="stage")
                        oh_sb = ohpool.tile([D, max_ohC, TILE], BF16, tag="oh")
                        # per-tile one-hots over each tile's chunk span,
                        # shifted so group offsets land on 0..127
                        oc = 0
                        ocol = {}
                        for (tr, kmin, nk) in sched:
                            ocol[tr] = oc
                            nc.vector.scalar_tensor_tensor(
                                out=oh_sb[:, oc:oc + nk, :],
                                in0=do_sb[:, gco + kmin:gco + kmin + nk]
                                    .unsqueeze(2).broadcast_to([D, nk, TILE]),
                                scalar=float(-TILE * tr),
                                in1=iota.unsqueeze(1).broadcast_to([D, nk, TILE]),
                                op0=ALU.add,
                                op1=ALU.is_equal,
                            )
                            oc += nk
                        for c0 in range(0, Cgb, GMAX):
                            cn = min(GMAX, Cgb - c0)
                            nc.gpsimd.dma_gather(
                                stg[:, c0:c0 + cn, :],
                                src_full[plan.blo[b]:plan.bhi[b], :],
                                idx_sb[:, ic0 + c0 * 8:ic0 + (c0 + cn) * 8],
                                cn * TILE,
                                cn * TILE,
                                D,
                                queue_num=qcnt[0] % NQ,
                            )
                            qcnt[0] += 1
                        for (tr, kmin, nk) in sched:
                            t = tiles[tr]
                            for j in range(nk):
                                nc.tensor.matmul(
                                    accs[t][:], lhsT=oh_sb[:, ocol[tr] + j, :],
                                    rhs=stg[:, kmin + j, :],
                                    start=(t not in started), stop=False)
                                started.add(t)
                        gco += Cgb

                    for t in tiles:
                        if t not in started:
                            continue
                        w = tw(t)
                        acc = accs[t]
                        # bias as rank-1: outer(sqrt(deg), b); sdeg rows
                        # beyond the tile width are zero-padded on the host
                        nc.tensor.matmul(
                            acc[:],
                            lhsT=sd_sb[:, t * TILE:(t + 1) * TILE],
                            rhs=br_sb[:, li * D:(li + 1) * D],
                            start=False, stop=True)
                        if li == 0 and stage == "l1":
                            ev = evpool.tile([TILE, D], F32, tag="ev")
                            nc.scalar.activation(ev[:w, :], acc[:w, :], AF.Copy,
                                                 scale=dv_sb[:w, t:t + 1])
                            nc.sync.dma_start(
                                out=out_d[t * TILE:t * TILE + w, :],
                                in_=ev[:w, :])
                        elif li == 0:
                            # fused layer-2 dense: h1' = (out1 @ W2) * dinv
                            ev = evpool.tile([TILE, D], BF16, tag="ev")
                            nc.scalar.activation(ev[:w, :], acc[:w, :], AF.Copy,
                                                 scale=dv_sb[:w, t:t + 1])
                            ptr = trpool.tile([D, TILE], BF16, tag="ptr")
                            nc.tensor.transpose(ptr[:, :w], ev[:w, :],
                                                ident[:w, :w])
                            trs = evpool.tile([D, TILE], BF16, tag="trs")
                            nc.vector.tensor_copy(trs[:, :w], ptr[:, :w])
                            pd = pdpool.tile([TILE, D], F32, tag="pd")
                            nc.tensor.matmul(pd[:w, :], lhsT=trs[:, :w], rhs=W2,
                                             start=True, stop=True)
                            hs1 = hself[1][:, t * D:(t + 1) * D]
                            nc.scalar.activation(hs1[:w, :], pd[:w, :], AF.Copy,
                                                 scale=dv_sb[:w, t:t + 1])
                            nc.sync.dma_start(
                                out=hb[1][t * TILE:t * TILE + w, :],
                                in_=hs1[:w, :])
                        else:
                            ev = evpool.tile([TILE, D], F32, tag="ev")
                            nc.scalar.activation(ev[:w, :], acc[:w, :], AF.Copy,
                                                 scale=dv_sb[:w, t:t + 1])
                            nc.sync.dma_start(
                                out=out_d[t * TILE:t * TILE + w, :],
                                in_=ev[:w, :])

            def allgather_split(li):
                # four tile-aligned chunks -> hf in chunk-major layout, one
                # chunk per gather src-block (indices are built against it)
                for jc in range(4):
                    lo = plan.ag_base[jc]
                    hi = lo + plan.ag_szs[jc]
                    nc.gpsimd.collective_compute(
                        "AllGather", ALU.bypass,
                        replica_groups=[list(range(NCORES))],
                        ins=[hb[li][lo:hi, :].opt()],
                        outs=[hf[li][plan.blo[jc]:plan.bhi[jc], :].opt()])

            if stage != "dense":
                allgather_split(0)
                sparse_layer(0)
            if stage == "l1d":
                for t in range(nt):
                    w = tw(t)
                    ev3 = evpool.tile([TILE, D], BF16, tag="ev3")
                    nc.sync.dma_start(out=ev3[:w, :],
                                      in_=hb[1][t * TILE:t * TILE + w, :])
                    ev3f = evpool.tile([TILE, D], F32, tag="ev3f")
                    nc.scalar.activation(ev3f[:w, :], ev3[:w, :], AF.Copy)
                    nc.sync.dma_start(out=out_d[t * TILE:t * TILE + w, :],
                                      in_=ev3f[:w, :])
            if stage == "full":
                allgather_split(1)
                sparse_layer(1)

    nc.compile()
    return nc


def _install_ntff_hook():
    """antenv.axon_hooks is absent in this image; synthesize it and register
    the ctypes NTFF profile hook from the boot module."""
    import types
    if "antenv.axon_hooks" in sys.modules:
        return
    try:
        from trn_agent_boot.trn_boot import _ntff_profile_via_ctypes
        hook = _ntff_profile_via_ctypes("/opt/axon/libaxon_pjrt.so")
    except Exception as e:
        print(f"[kernel] ntff hook unavailable: {e}", flush=True)
        hook = None
    mod = types.ModuleType("antenv.axon_hooks")
    mod._hook = hook
    mod.set_axon_ntff_profile_hook = lambda h: setattr(mod, "_hook", h)
    mod.get_axon_ntff_profile_hook = lambda: mod._hook
    sys.modules["antenv.axon_hooks"] = mod
    import antenv
    antenv.axon_hooks = mod


def _run(plan, x, W1, b1, W2, b2, trace=False, stage="full"):
    import time
    if trace:
        _install_ntff_hook()
    t0 = time.time()
    nc = _build(plan, stage=stage)
    t1 = time.time()
    if os.environ.get("GCN_VERBOSE"):
        print(f"[kernel] build+compile: {t1 - t0:.1f}s", flush=True)
    ns, nt = plan.ns, plan.nt
    iota_t = np.tile(np.arange(TILE, dtype=np.float32), (TILE, 1))
    ident_t = np.eye(TILE, dtype=np.float32)
    consts = np.concatenate([iota_t, ident_t], axis=1).astype(NPBF16)
    iota16 = iota_t.astype(np.float16)
    wts = np.concatenate([W1.astype(np.float32), W2.astype(np.float32)],
                         axis=1).astype(NPBF16)
    brow = np.concatenate([b1.astype(np.float32), b2.astype(np.float32)]
                          ).reshape(1, 2 * D).astype(NPBF16)

    in_maps = []
    for c in range(NCORES):
        lo, hi = c * ns, (c + 1) * ns
        dv = plan.dinv[lo:hi]
        # column t of dcol holds dinv[lo + t*128 : lo + (t+1)*128] (pad 1.0)
        dcol = np.ones((nt, TILE), dtype=np.float32)
        dcol.reshape(-1)[:ns] = dv
        dcol = np.ascontiguousarray(dcol.T)
        sdr = np.zeros((1, nt * TILE), dtype=np.float32)
        sdr[0, :ns] = plan.sdeg[lo:hi]
        idx, doff = plan.core_inputs(c)
        in_maps.append({
            "xT": np.ascontiguousarray(
                (x[lo:hi].astype(np.float32) * dv[:, None]).T).astype(NPBF16),
            "wts": wts, "consts": consts, "brow": brow, "iota16": iota16,
            "dinv_c": dcol, "sdeg_r": sdr.astype(NPBF16),
            "idx": idx, "doff": doff,
        })
    t2 = time.time()
    res = run_bass_kernel_spmd(nc, in_maps, core_ids=list(range(NCORES)),
                               trace=trace)
    if os.environ.get("GCN_VERBOSE"):
        print(f"[kernel] prep inputs: {t2 - t1:.1f}s, run: {time.time() - t2:.1f}s",
              flush=True)
    out = np.concatenate([res.results[c]["out"] for c in range(NCORES)], axis=0)
    return out, res


def kernel(x, edge_index, W1, b1, W2, b2):
    plan = Plan(x.shape[0], np.asarray(edge_index))
    out, _ = _run(plan, np.asarray(x), np.asarray(W1), np.asarray(b1),
                  np.asarray(W2), np.asarray(b2))
    return out



# revision 45
# speedup vs baseline: 1.0386x; 1.0386x over previous
"""Trainium2 Bass kernel: 2-layer GCN (PyG-style GCNConv x2) on 8 NeuronCores.

Strategy:
  - Nodes sharded contiguously across 8 cores (12500 rows each).
  - Per layer: dense h' = (x @ W) * dinv[row] computed on the owning core,
    AllGather h' to every core (51MB replica), then per-core sparse
    aggregation over its in-edges:
      gather h'[src] rows via dma_gather (int16 idx -> 4 src blocks of 25000),
      scatter-add via one-hot matmul into PSUM per 128-dst tile,
      bias added as rank-1 matmul outer(sqrt(deg), b),
      eviction scaled by dinv[dst] on the scalar engine.
  - The per-edge norm dinv[src]*dinv[dst] is folded into the two node-level
    scalings, so no per-edge multiply exists anywhere.
"""

import os
import sys

for _p in ("/opt/trn_rl_repo",):
    if _p not in sys.path:
        sys.path.append(_p)

import numpy as np
import ml_dtypes

import concourse.bacc as bacc
import concourse.bass as bass
import concourse.mybir as mybir
import concourse.tile as tile
from concourse.bass_utils import run_bass_kernel_spmd

F32 = mybir.dt.float32
BF16 = mybir.dt.bfloat16
F16 = mybir.dt.float16
I16 = mybir.dt.int16
AF = mybir.ActivationFunctionType
ALU = mybir.AluOpType
NPBF16 = ml_dtypes.bfloat16
NQ = 4  # SWDGE queues (ring-drain parallelism for dma_gather)

N_NODES = 100000
D = 128
NCORES = 8
TILE = 128


def _ceil_div(a, b):
    return (a + b - 1) // b


class Plan:
    """Core-uniform structure tables derived from the edge index.

    Chunks are laid out per (tile-group, src-block) cell: within a cell a
    core's edges (sorted by dst tile, then src) fill C_gb*128 slots where
    C_gb = max over cores of ceil(count/128). A chunk may span a tile
    boundary; doff holds the GROUP-relative dst offset (0..G*128-1), and
    the per-tile one-hot is built with a -128*t_rel shift so out-of-tile
    slots contribute zero. The (tile -> chunk range) schedule is the union
    over cores, so one SPMD program fits all cores.
    """

    def __init__(self, n_nodes, edge_index, group_tiles=4):
        self.n = n_nodes
        self.ns = n_nodes // NCORES            # nodes per core
        self.nt = _ceil_div(self.ns, TILE)     # dst tiles per core
        self.last_w = self.ns - (self.nt - 1) * TILE
        self.G = group_tiles

        # deg includes self-loops (as the reference adds them), but the
        # self-loop edges themselves are NOT gathered: their contribution
        # h'[dst] is added locally from the dense-phase output via one
        # identity matmul per tile.
        src = np.asarray(edge_index[0])
        dst = np.asarray(edge_index[1])
        deg = (np.bincount(dst, minlength=n_nodes) + 1).astype(np.float32)
        self.dinv = deg ** -0.5
        self.sdeg = np.sqrt(deg)

        # tile groups
        self.groups = [list(range(g0, min(g0 + self.G, self.nt)))
                       for g0 in range(0, self.nt, self.G)]
        ng = len(self.groups)

        # The AllGather is split into 4 tile-aligned chunks, one per gather
        # src-block, so each block's gathers unblock as its chunk lands.
        # Chunk outputs are core-major, so the replica hf holds row i of the
        # original numbering at remapped position frow(i); all gather indices
        # are built against that layout.
        tchunk = [25, 25, 24, self.nt - 74]     # tiles per chunk
        szs, base = [], []
        o = 0
        for k, tc_ in enumerate(tchunk):
            base.append(o)
            if k < 3:
                szs.append(tc_ * TILE)
                o += tc_ * TILE
            else:
                szs.append(self.ns - o)
        self.ag_szs, self.ag_base = szs, base   # per-core chunk row ranges
        self.blo = []
        self.bhi = []
        o = 0
        for s in szs:
            self.blo.append(o)
            self.bhi.append(o + NCORES * s)
            o += NCORES * s
        assert o == n_nodes
        assert max(h - l for l, h in zip(self.blo, self.bhi)) < 32768
        self.nblk = len(self.blo)
        nodes = np.arange(n_nodes, dtype=np.int64)
        r = nodes % self.ns
        j = np.searchsorted(np.asarray(base), r, side="right") - 1
        szs_a = np.asarray(szs)[j]
        self.frow = (np.asarray(self.blo)[j] + (nodes // self.ns) * szs_a
                     + r - np.asarray(base)[j])

        core = dst // self.ns
        dstloc = dst % self.ns
        tloc = dstloc // TILE
        gi = np.minimum(tloc // self.G, ng - 1)
        goff = dstloc - gi * self.G * TILE     # group-relative dst offset
        frow_e = self.frow[src]
        blk = np.searchsorted(np.asarray(self.blo), frow_e, side="right") - 1
        cell = (core * ng + gi) * self.nblk + blk
        # sort: cell, then dst tile, then remapped src row (ascending HBM)
        order = np.lexsort((frow_e, tloc, cell))
        self.src_s = frow_e[order]
        self.goff_s = goff[order]
        ncell = NCORES * ng * self.nblk
        cnt3 = np.bincount(cell, minlength=ncell).reshape(
            NCORES, ng, self.nblk)
        self.cnt3 = cnt3
        self.seg_off = np.zeros(ncell + 1, dtype=np.int64)
        np.cumsum(cnt3.reshape(-1), out=self.seg_off[1:])

        # per (core, g, blk, t_rel) counts -> union (tile -> chunk range)
        cell4 = cell * self.G + (tloc - gi * self.G)
        cnt4 = np.bincount(cell4, minlength=ncell * self.G).reshape(
            NCORES, ng, self.nblk, self.G)
        pre4 = np.cumsum(cnt4, axis=3) - cnt4   # exclusive prefix within cell

        # chunks per (g, b): cross-core max
        self.gb_C = _ceil_div(cnt3, TILE).max(axis=0)  # [ng, nblk]

        # schedule[g][b] -> list of (t_rel, kmin, nk)
        self.sched = []
        for g in range(ng):
            row = []
            for b in range(self.nblk):
                ent = []
                for tr in range(len(self.groups[g])):
                    c4 = cnt4[:, g, b, tr]
                    if not c4.any():
                        continue
                    p4 = pre4[:, g, b, tr]
                    alive = c4 > 0
                    kmin = int((p4[alive] // TILE).min())
                    kmax = int(((p4[alive] + c4[alive] - 1) // TILE).max())
                    ent.append((tr, kmin, kmax - kmin + 1))
                row.append(ent)
            self.sched.append(row)

        # column offsets in the concatenated idx / dstoff DRAM buffers
        self.idx_col = []     # [g][b] -> start col in idx buffer (int16 wrap)
        self.ch_col = []      # [g] -> start chunk col in dstoff buffer
        ic = 0
        cc = 0
        for g in range(ng):
            self.ch_col.append(cc)
            row = []
            for b in range(self.nblk):
                row.append(ic)
                ic += int(self.gb_C[g, b]) * (TILE // 16)
                cc += int(self.gb_C[g, b])
            self.idx_col.append(row)
        self.idx_cols = ic
        self.ch_cols = cc

    def core_inputs(self, c):
        """Build idx (int16 [128, idx_cols]) and dstoff (fp16 [128, ch_cols])."""
        ng = len(self.groups)
        idx = np.zeros((16, self.idx_cols), dtype=np.int16)
        doff = np.full((128, self.ch_cols), -1.0, dtype=np.float16)
        for g in range(ng):
            ch = self.ch_col[g]
            for b in range(self.nblk):
                icol = self.idx_col[g][b]
                nch = int(self.gb_C[g, b])
                cnt = int(self.cnt3[c, g, b])
                o = self.seg_off[(c * ng + g) * self.nblk + b]
                nslots = nch * TILE
                a = np.zeros(nslots, dtype=np.int16)
                a[:cnt] = (self.src_s[o:o + cnt] - self.blo[b]).astype(np.int16)
                idx[:, icol:icol + nch * 8] = a.reshape(nch * 8, 16).T
                dv = np.full(nslots, -1.0, dtype=np.float16)
                dv[:cnt] = self.goff_s[o:o + cnt].astype(np.float16)
                doff[:, ch:ch + nch] = dv.reshape(nch, 128).T
                ch += nch
        idx_full = np.tile(idx, (8, 1))
        return idx_full, doff


def _build(plan, stage="full"):
    """Build the SPMD bass program (shared by all 8 cores)."""
    n, ns, nt, nblk = plan.n, plan.ns, plan.nt, plan.nblk
    nc = bacc.Bacc("TRN2", target_bir_lowering=False, debug=False,
                   num_devices=NCORES, num_swdge_queues=NQ)

    xT = nc.dram_tensor("xT", [D, ns], BF16, kind="ExternalInput").ap()
    wts = nc.dram_tensor("wts", [D, 2 * D], BF16, kind="ExternalInput").ap()
    consts = nc.dram_tensor("consts", [D, 2 * D], BF16, kind="ExternalInput").ap()
    brow = nc.dram_tensor("brow", [1, 2 * D], BF16, kind="ExternalInput").ap()
    dinv_c = nc.dram_tensor("dinv_c", [D, nt], F32, kind="ExternalInput").ap()
    sdeg_r = nc.dram_tensor("sdeg_r", [1, nt * TILE], BF16, kind="ExternalInput").ap()
    idx_d = nc.dram_tensor("idx", [D, plan.idx_cols], I16, kind="ExternalInput").ap()
    doff_d = nc.dram_tensor("doff", [D, plan.ch_cols], F16, kind="ExternalInput").ap()
    iota16_d = nc.dram_tensor("iota16", [D, D], F16, kind="ExternalInput").ap()
    out_d = nc.dram_tensor("out", [ns, D], F32, kind="ExternalOutput").ap()

    hb = [nc.dram_tensor(f"h{i}b", [ns, D], BF16).ap() for i in range(2)]
    hf = [nc.dram_tensor(f"h{i}f", [n, D], BF16, addr_space="Shared").ap()
          for i in range(2)]

    max_C = max(int(plan.gb_C[g].sum()) for g in range(len(plan.groups)))
    max_icols = max_C * 8
    # one-hot columns per (g, b): sum over scheduled tiles of their k-span
    max_ohC = max(sum(nk for (_, _, nk) in plan.sched[g][b])
                  for g in range(len(plan.groups))
                  for b in range(plan.nblk))

    with tile.TileContext(nc) as tc:
        with (
            tc.tile_pool(name="const", bufs=1) as cpool,
            tc.tile_pool(name="xstream", bufs=3) as xpool,
            tc.tile_pool(name="stage", bufs=4) as spool,
            tc.tile_pool(name="oh", bufs=4) as ohpool,
            tc.tile_pool(name="aux", bufs=3) as auxpool,
            tc.tile_pool(name="ev", bufs=4) as evpool,
            tc.tile_pool(name="acc", bufs=5, space="PSUM") as accpool,
            tc.tile_pool(name="ptr", bufs=1, space="PSUM") as trpool,
            tc.tile_pool(name="pd", bufs=2, space="PSUM") as pdpool,
        ):
            w_sb = cpool.tile([D, 2 * D], BF16, tag="w")
            nc.sync.dma_start(out=w_sb[:], in_=wts[:])
            co_sb = cpool.tile([D, 2 * D], BF16, tag="co")
            nc.sync.dma_start(out=co_sb[:], in_=consts[:])
            br_sb = cpool.tile([1, 2 * D], BF16, tag="br")
            nc.sync.dma_start(out=br_sb[:], in_=brow[:])
            dv_sb = cpool.tile([D, nt], F32, tag="dv")
            nc.sync.dma_start(out=dv_sb[:], in_=dinv_c[:])
            sd_sb = cpool.tile([1, nt * TILE], BF16, tag="sd")
            nc.sync.dma_start(out=sd_sb[:], in_=sdeg_r[:])

            io_sb = cpool.tile([D, D], F16, tag="io16")
            nc.sync.dma_start(out=io_sb[:], in_=iota16_d[:])

            W1 = w_sb[:, 0:D]
            W2 = w_sb[:, D:2 * D]
            iota = io_sb[:, 0:D]
            ident = co_sb[:, D:2 * D]

            # dense outputs h{0,1}' stay resident: the self-loop term is
            # added from here straight into PSUM (no gather round-trip)
            hself = [cpool.tile([TILE, nt * D], BF16, tag=f"hself{i}",
                                name=f"hself{i}")
                     for i in range(2)]

            def tw(t):
                return TILE if t < nt - 1 else plan.last_w

            # ---- layer-1 dense: h0' = (x @ W1) * dinv ----
            XB = 8  # xT tiles per DMA batch
            for t0 in range(0, nt, XB):
                tn = min(XB, nt - t0)
                wlast = tw(t0 + tn - 1)
                cols = (tn - 1) * TILE + wlast
                xt_t = xpool.tile([D, XB * TILE], BF16, tag="xt")
                nc.sync.dma_start(out=xt_t[:, :cols],
                                  in_=xT[:, t0 * TILE:t0 * TILE + cols])
                for t in range(t0, t0 + tn):
                    w = tw(t)
                    pd = pdpool.tile([TILE, D], F32, tag="pd")
                    nc.tensor.matmul(
                        pd[:w, :],
                        lhsT=xt_t[:, (t - t0) * TILE:(t - t0) * TILE + w],
                        rhs=W1, start=True, stop=True)
                    # dinv[row] is folded into xT on the host; plain copy,
                    # alternating engines to halve the eviction chain
                    hs = hself[0][:, t * D:(t + 1) * D]
                    if t % 2 == 0:
                        nc.scalar.activation(hs[:w, :], pd[:w, :], AF.Copy)
                    else:
                        nc.vector.tensor_copy(hs[:w, :], pd[:w, :])
                    nc.sync.dma_start(out=hb[0][t * TILE:t * TILE + w, :],
                                      in_=hs[:w, :])

            if stage == "dense":
                nc.gpsimd.collective_compute(
                    "AllGather", ALU.bypass,
                    replica_groups=[list(range(NCORES))],
                    ins=[hb[0].opt()], outs=[hf[0].opt()])
                for t in range(nt):
                    w = tw(t)
                    ev = evpool.tile([TILE, D], BF16, tag="ev")
                    nc.sync.dma_start(out=ev[:w, :],
                                      in_=hf[0][t * TILE:t * TILE + w, :])
                    evf = evpool.tile([TILE, D], F32, tag="evf")
                    nc.scalar.activation(evf[:w, :], ev[:w, :], AF.Copy)
                    nc.sync.dma_start(out=out_d[t * TILE:t * TILE + w, :],
                                      in_=evf[:w, :])

            # ---- sparse layer (templated over layer index) ----
            max_Cgb = max((int(plan.gb_C[g, b]) for g in range(len(plan.groups))
                           for b in range(nblk)), default=1)
            GMAX = 8  # dma_gather caps at ~1024 idxs (16KB desc ring)

            qcnt = [0]  # global dma_gather round-robin across SWDGE queues

            def sparse_layer(li):
                src_full = hf[li]
                for g, tiles in enumerate(plan.groups):
                    Ctot = int(plan.gb_C[g].sum())
                    if Ctot == 0:
                        continue
                    icols = Ctot * 8
                    idx_sb = auxpool.tile([D, max_icols], I16, tag="idx")
                    nc.sync.dma_start(
                        out=idx_sb[:, :icols],
                        in_=idx_d[:, plan.idx_col[g][0]:plan.idx_col[g][0] + icols])
                    do_sb = auxpool.tile([D, max_C], F16, tag="doff")
                    nc.sync.dma_start(
                        out=do_sb[:, :Ctot],
                        in_=doff_d[:, plan.ch_col[g]:plan.ch_col[g] + Ctot])

                    accs = {}
                    started = set()
                    for t in tiles:
                        accs[t] = accpool.tile([TILE, D], F32, tag="acc", name=f"acc_t{t}")
                        # self-loop term: acc[dst] += h'[dst]
                        w = tw(t)
                        nc.tensor.matmul(
                            accs[t][:w, :], lhsT=ident[:w, :w],
                            rhs=hself[li][:w, t * D:(t + 1) * D],
                            start=True, stop=False)
                        started.add(t)

                    gco = 0  # chunk offset within the group's doff columns
                    for b in range(nblk):
                        Cgb = int(plan.gb_C[g, b])
                        if Cgb == 0:
                            continue
                        sched = plan.sched[g][b]
                        ic0 = plan.idx_col[g][b] - plan.idx_col[g][0]
                        stg = spool.tile([D, max_Cgb, TILE], BF16, tag="stage")
                        oh_sb = ohpool.tile([D, max_ohC, TILE], BF16, tag="oh")
                        # per-tile one-hots over each tile's chunk span,
                        # shifted so group offsets land on 0..127
                        oc = 0
                        ocol = {}
                        for (tr, kmin, nk) in sched:
                            ocol[tr] = oc
                            nc.vector.scalar_tensor_tensor(
                                out=oh_sb[:, oc:oc + nk, :],
                                in0=do_sb[:, gco + kmin:gco + kmin + nk]
                                    .unsqueeze(2).broadcast_to([D, nk, TILE]),
                                scalar=float(-TILE * tr),
                                in1=iota.unsqueeze(1).broadcast_to([D, nk, TILE]),
                                op0=ALU.add,
                                op1=ALU.is_equal,
                            )
                            oc += nk
                        for c0 in range(0, Cgb, GMAX):
                            cn = min(GMAX, Cgb - c0)
                            nc.gpsimd.dma_gather(
                                stg[:, c0:c0 + cn, :],
                                src_full[plan.blo[b]:plan.bhi[b], :],
                                idx_sb[:, ic0 + c0 * 8:ic0 + (c0 + cn) * 8],
                                cn * TILE,
                                cn * TILE,
                                D,
                                queue_num=qcnt[0] % NQ,
                            )
                            qcnt[0] += 1
                        for (tr, kmin, nk) in sched:
                            t = tiles[tr]
                            for j in range(nk):
                                nc.tensor.matmul(
                                    accs[t][:], lhsT=oh_sb[:, ocol[tr] + j, :],
                                    rhs=stg[:, kmin + j, :],
                                    start=(t not in started), stop=False)
                                started.add(t)
                        gco += Cgb

                    for t in tiles:
                        if t not in started:
                            continue
                        w = tw(t)
                        acc = accs[t]
                        # bias as rank-1: outer(sqrt(deg), b); sdeg rows
                        # beyond the tile width are zero-padded on the host
                        nc.tensor.matmul(
                            acc[:],
                            lhsT=sd_sb[:, t * TILE:(t + 1) * TILE],
                            rhs=br_sb[:, li * D:(li + 1) * D],
                            start=False, stop=True)
                        if li == 0 and stage == "l1":
                            ev = evpool.tile([TILE, D], F32, tag="ev")
                            nc.scalar.activation(ev[:w, :], acc[:w, :], AF.Copy,
                                                 scale=dv_sb[:w, t:t + 1])
                            nc.sync.dma_start(
                                out=out_d[t * TILE:t * TILE + w, :],
                                in_=ev[:w, :])
                        elif li == 0:
                            # fused layer-2 dense: h1' = (out1 @ W2) * dinv
                            ev = evpool.tile([TILE, D], BF16, tag="ev")
                            nc.scalar.activation(ev[:w, :], acc[:w, :], AF.Copy,
                                                 scale=dv_sb[:w, t:t + 1])
                            ptr = trpool.tile([D, TILE], BF16, tag="ptr")
                            nc.tensor.transpose(ptr[:, :w], ev[:w, :],
                                                ident[:w, :w])
                            trs = evpool.tile([D, TILE], BF16, tag="trs")
                            nc.vector.tensor_copy(trs[:, :w], ptr[:, :w])
                            pd = pdpool.tile([TILE, D], F32, tag="pd")
                            nc.tensor.matmul(pd[:w, :], lhsT=trs[:, :w], rhs=W2,
                                             start=True, stop=True)
                            hs1 = hself[1][:, t * D:(t + 1) * D]
                            nc.scalar.activation(hs1[:w, :], pd[:w, :], AF.Copy,
                                                 scale=dv_sb[:w, t:t + 1])
                            nc.sync.dma_start(
                                out=hb[1][t * TILE:t * TILE + w, :],
                                in_=hs1[:w, :])
                        else:
                            ev = evpool.tile([TILE, D], F32, tag="ev")
                            nc.scalar.activation(ev[:w, :], acc[:w, :], AF.Copy,
                                                 scale=dv_sb[:w, t:t + 1])
                            nc.sync.dma_start(
                                out=out_d[t * TILE:t * TILE + w, :],
                                in_=ev[:w, :])

            def allgather_split(li):
                # four tile-aligned chunks -> hf in chunk-major layout, one
                # chunk per gather src-block (indices are built against it)
                for jc in range(4):
                    lo = plan.ag_base[jc]
                    hi = lo + plan.ag_szs[jc]
                    nc.gpsimd.collective_compute(
                        "AllGather", ALU.bypass,
                        replica_groups=[list(range(NCORES))],
                        ins=[hb[li][lo:hi, :].opt()],
                        outs=[hf[li][plan.blo[jc]:plan.bhi[jc], :].opt()])

            if stage != "dense":
                allgather_split(0)
                sparse_layer(0)
            if stage == "l1d":
                for t in range(nt):
                    w = tw(t)
                    ev3 = evpool.tile([TILE, D], BF16, tag="ev3")
                    nc.sync.dma_start(out=ev3[:w, :],
                                      in_=hb[1][t * TILE:t * TILE + w, :])
                    ev3f = evpool.tile([TILE, D], F32, tag="ev3f")
                    nc.scalar.activation(ev3f[:w, :], ev3[:w, :], AF.Copy)
                    nc.sync.dma_start(out=out_d[t * TILE:t * TILE + w, :],
                                      in_=ev3f[:w, :])
            if stage == "full":
                allgather_split(1)
                sparse_layer(1)

    nc.compile()
    return nc


def _install_ntff_hook():
    """antenv.axon_hooks is absent in this image; synthesize it and register
    the ctypes NTFF profile hook from the boot module."""
    import types
    if "antenv.axon_hooks" in sys.modules:
        return
    try:
        from trn_agent_boot.trn_boot import _ntff_profile_via_ctypes
        hook = _ntff_profile_via_ctypes("/opt/axon/libaxon_pjrt.so")
    except Exception as e:
        print(f"[kernel] ntff hook unavailable: {e}", flush=True)
        hook = None
    mod = types.ModuleType("antenv.axon_hooks")
    mod._hook = hook
    mod.set_axon_ntff_profile_hook = lambda h: setattr(mod, "_hook", h)
    mod.get_axon_ntff_profile_hook = lambda: mod._hook
    sys.modules["antenv.axon_hooks"] = mod
    import antenv
    antenv.axon_hooks = mod


def _run(plan, x, W1, b1, W2, b2, trace=False, stage="full"):
    import time
    if trace:
        _install_ntff_hook()
    t0 = time.time()
    nc = _build(plan, stage=stage)
    t1 = time.time()
    if os.environ.get("GCN_VERBOSE"):
        print(f"[kernel] build+compile: {t1 - t0:.1f}s", flush=True)
    ns, nt = plan.ns, plan.nt
    iota_t = np.tile(np.arange(TILE, dtype=np.float32), (TILE, 1))
    ident_t = np.eye(TILE, dtype=np.float32)
    consts = np.concatenate([iota_t, ident_t], axis=1).astype(NPBF16)
    iota16 = iota_t.astype(np.float16)
    wts = np.concatenate([W1.astype(np.float32), W2.astype(np.float32)],
                         axis=1).astype(NPBF16)
    brow = np.concatenate([b1.astype(np.float32), b2.astype(np.float32)]
                          ).reshape(1, 2 * D).astype(NPBF16)

    in_maps = []
    for c in range(NCORES):
        lo, hi = c * ns, (c + 1) * ns
        dv = plan.dinv[lo:hi]
        # column t of dcol holds dinv[lo + t*128 : lo + (t+1)*128] (pad 1.0)
        dcol = np.ones((nt, TILE), dtype=np.float32)
        dcol.reshape(-1)[:ns] = dv
        dcol = np.ascontiguousarray(dcol.T)
        sdr = np.zeros((1, nt * TILE), dtype=np.float32)
        sdr[0, :ns] = plan.sdeg[lo:hi]
        idx, doff = plan.core_inputs(c)
        in_maps.append({
            "xT": np.ascontiguousarray(
                (x[lo:hi].astype(np.float32) * dv[:, None]).T).astype(NPBF16),
            "wts": wts, "consts": consts, "brow": brow, "iota16": iota16,
            "dinv_c": dcol, "sdeg_r": sdr.astype(NPBF16),
            "idx": idx, "doff": doff,
        })
    t2 = time.time()
    res = run_bass_kernel_spmd(nc, in_maps, core_ids=list(range(NCORES)),
                               trace=trace)
    if os.environ.get("GCN_VERBOSE"):
        print(f"[kernel] prep inputs: {t2 - t1:.1f}s, run: {time.time() - t2:.1f}s",
              flush=True)
    out = np.concatenate([res.results[c]["out"] for c in range(NCORES)], axis=0)
    return out, res


def kernel(x, edge_index, W1, b1, W2, b2):
    plan = Plan(x.shape[0], np.asarray(edge_index))
    out, _ = _run(plan, np.asarray(x), np.asarray(W1), np.asarray(b1),
                  np.asarray(W2), np.asarray(b2))
    return out



# revision 60
# speedup vs baseline: 1.0536x; 1.0145x over previous
"""Trainium2 Bass kernel: 2-layer GCN (PyG-style GCNConv x2) on 8 NeuronCores.

Strategy:
  - Nodes sharded contiguously across 8 cores (12500 rows each).
  - Per layer: dense h' = (x @ W) * dinv[row] computed on the owning core,
    AllGather h' to every core (51MB replica), then per-core sparse
    aggregation over its in-edges:
      gather h'[src] rows via dma_gather (int16 idx -> 4 src blocks of 25000),
      scatter-add via one-hot matmul into PSUM per 128-dst tile,
      bias added as rank-1 matmul outer(sqrt(deg), b),
      eviction scaled by dinv[dst] on the scalar engine.
  - The per-edge norm dinv[src]*dinv[dst] is folded into the two node-level
    scalings, so no per-edge multiply exists anywhere.
"""

import os
import sys

for _p in ("/opt/trn_rl_repo",):
    if _p not in sys.path:
        sys.path.append(_p)

import numpy as np
import ml_dtypes

import concourse.bacc as bacc
import concourse.bass as bass
import concourse.mybir as mybir
import concourse.tile as tile
from concourse.bass_utils import run_bass_kernel_spmd

F32 = mybir.dt.float32
BF16 = mybir.dt.bfloat16
F16 = mybir.dt.float16
I16 = mybir.dt.int16
AF = mybir.ActivationFunctionType
ALU = mybir.AluOpType
NPBF16 = ml_dtypes.bfloat16
NQ = 4  # SWDGE queues (ring-drain parallelism for dma_gather)

N_NODES = 100000
D = 128
NCORES = 8
TILE = 128


def _ceil_div(a, b):
    return (a + b - 1) // b


class Plan:
    """Core-uniform structure tables derived from the edge index.

    Chunks are laid out per (tile-group, src-block) cell: within a cell a
    core's edges (sorted by dst tile, then src) fill C_gb*128 slots where
    C_gb = max over cores of ceil(count/128). A chunk may span a tile
    boundary; doff holds the GROUP-relative dst offset (0..G*128-1), and
    the per-tile one-hot is built with a -128*t_rel shift so out-of-tile
    slots contribute zero. The (tile -> chunk range) schedule is the union
    over cores, so one SPMD program fits all cores.
    """

    def __init__(self, n_nodes, edge_index, group_tiles=4):
        self.n = n_nodes
        self.ns = n_nodes // NCORES            # nodes per core
        self.nt = _ceil_div(self.ns, TILE)     # dst tiles per core
        self.last_w = self.ns - (self.nt - 1) * TILE
        self.G = group_tiles

        # deg includes self-loops (as the reference adds them), but the
        # self-loop edges themselves are NOT gathered: their contribution
        # h'[dst] is added locally from the dense-phase output via one
        # identity matmul per tile.
        src = np.asarray(edge_index[0])
        dst = np.asarray(edge_index[1])
        deg = (np.bincount(dst, minlength=n_nodes) + 1).astype(np.float32)
        self.dinv = deg ** -0.5
        self.sdeg = np.sqrt(deg)

        # tile groups
        self.groups = [list(range(g0, min(g0 + self.G, self.nt)))
                       for g0 in range(0, self.nt, self.G)]
        ng = len(self.groups)

        # The AllGather is split into 4 tile-aligned chunks, one per gather
        # src-block, so each block's gathers unblock as its chunk lands.
        # Chunk outputs are core-major, so the replica hf holds row i of the
        # original numbering at remapped position frow(i); all gather indices
        # are built against that layout.
        tchunk = [25, 25, 24, self.nt - 74]     # tiles per chunk
        szs, base = [], []
        o = 0
        for k, tc_ in enumerate(tchunk):
            base.append(o)
            if k < 3:
                szs.append(tc_ * TILE)
                o += tc_ * TILE
            else:
                szs.append(self.ns - o)
        self.ag_szs, self.ag_base = szs, base   # per-core chunk row ranges
        self.blo = []
        self.bhi = []
        o = 0
        for s in szs:
            self.blo.append(o)
            self.bhi.append(o + NCORES * s)
            o += NCORES * s
        assert o == n_nodes
        assert max(h - l for l, h in zip(self.blo, self.bhi)) < 32768
        self.nblk = len(self.blo)
        nodes = np.arange(n_nodes, dtype=np.int64)
        r = nodes % self.ns
        j = np.searchsorted(np.asarray(base), r, side="right") - 1
        szs_a = np.asarray(szs)[j]
        self.frow = (np.asarray(self.blo)[j] + (nodes // self.ns) * szs_a
                     + r - np.asarray(base)[j])

        core = dst // self.ns
        dstloc = dst % self.ns
        tloc = dstloc // TILE
        gi = np.minimum(tloc // self.G, ng - 1)
        goff = dstloc - gi * self.G * TILE     # group-relative dst offset
        frow_e = self.frow[src]
        blk = np.searchsorted(np.asarray(self.blo), frow_e, side="right") - 1
        cell = (core * ng + gi) * self.nblk + blk
        # sort: cell, then dst tile, then remapped src row (ascending HBM)
        order = np.lexsort((frow_e, tloc, cell))
        self.src_s = frow_e[order]
        self.goff_s = goff[order]
        ncell = NCORES * ng * self.nblk
        cnt3 = np.bincount(cell, minlength=ncell).reshape(
            NCORES, ng, self.nblk)
        self.cnt3 = cnt3
        self.seg_off = np.zeros(ncell + 1, dtype=np.int64)
        np.cumsum(cnt3.reshape(-1), out=self.seg_off[1:])

        # per (core, g, blk, t_rel) counts -> union (tile -> chunk range)
        cell4 = cell * self.G + (tloc - gi * self.G)
        cnt4 = np.bincount(cell4, minlength=ncell * self.G).reshape(
            NCORES, ng, self.nblk, self.G)
        pre4 = np.cumsum(cnt4, axis=3) - cnt4   # exclusive prefix within cell

        # chunks per (g, b): cross-core max
        self.gb_C = _ceil_div(cnt3, TILE).max(axis=0)  # [ng, nblk]

        # schedule[g][b] -> list of (t_rel, kmin, nk)
        self.sched = []
        for g in range(ng):
            row = []
            for b in range(self.nblk):
                ent = []
                for tr in range(len(self.groups[g])):
                    c4 = cnt4[:, g, b, tr]
                    if not c4.any():
                        continue
                    p4 = pre4[:, g, b, tr]
                    alive = c4 > 0
                    kmin = int((p4[alive] // TILE).min())
                    kmax = int(((p4[alive] + c4[alive] - 1) // TILE).max())
                    ent.append((tr, kmin, kmax - kmin + 1))
                row.append(ent)
            self.sched.append(row)

        # column offsets in the concatenated idx / dstoff DRAM buffers
        self.idx_col = []     # [g][b] -> start col in idx buffer (int16 wrap)
        self.ch_col = []      # [g] -> start chunk col in dstoff buffer
        ic = 0
        cc = 0
        for g in range(ng):
            self.ch_col.append(cc)
            row = []
            for b in range(self.nblk):
                row.append(ic)
                ic += int(self.gb_C[g, b]) * (TILE // 16)
                cc += int(self.gb_C[g, b])
            self.idx_col.append(row)
        self.idx_cols = ic
        self.ch_cols = cc
        # block-0 prefetch: the first groups' b=0 idx columns concatenated.
        # Their gathers depend only on the first AllGather chunk and are
        # issued up-front to hide the later chunks' latency. Capped so the
        # prefetch staging tile fits in SBUF.
        PF_CAP = 280          # chunks (~70KB/partition staging)
        self.pf_base = []     # [g] -> chunk offset in the prefetch buffer
        o = 0
        for g in range(ng):
            if o + int(self.gb_C[g, 0]) > PF_CAP:
                break
            self.pf_base.append(o)
            o += int(self.gb_C[g, 0])
        self.pf_ng = len(self.pf_base)   # groups covered by the prefetch
        self.pf_C = o         # total prefetched chunks
        self.idx0_cols = max(o * (TILE // 16), TILE // 16)

    def core_inputs(self, c):
        """Build idx/idx0 (int16) and dstoff (fp16 [128, ch_cols])."""
        ng = len(self.groups)
        idx = np.zeros((16, self.idx_cols), dtype=np.int16)
        idx0 = np.zeros((16, self.idx0_cols), dtype=np.int16)
        doff = np.full((128, self.ch_cols), -1.0, dtype=np.float16)
        for g in range(ng):
            ch = self.ch_col[g]
            for b in range(self.nblk):
                icol = self.idx_col[g][b]
                nch = int(self.gb_C[g, b])
                cnt = int(self.cnt3[c, g, b])
                o = self.seg_off[(c * ng + g) * self.nblk + b]
                nslots = nch * TILE
                a = np.zeros(nslots, dtype=np.int16)
                a[:cnt] = (self.src_s[o:o + cnt] - self.blo[b]).astype(np.int16)
                wrapped = a.reshape(nch * 8, 16).T
                idx[:, icol:icol + nch * 8] = wrapped
                if b == 0 and g < self.pf_ng:
                    p0 = self.pf_base[g] * 8
                    idx0[:, p0:p0 + nch * 8] = wrapped
                dv = np.full(nslots, -1.0, dtype=np.float16)
                dv[:cnt] = self.goff_s[o:o + cnt].astype(np.float16)
                doff[:, ch:ch + nch] = dv.reshape(nch, 128).T
                ch += nch
        return np.tile(idx, (8, 1)), np.tile(idx0, (8, 1)), doff


def _build(plan, stage="full"):
    """Build the SPMD bass program (shared by all 8 cores)."""
    n, ns, nt, nblk = plan.n, plan.ns, plan.nt, plan.nblk
    nc = bacc.Bacc("TRN2", target_bir_lowering=False, debug=False,
                   num_devices=NCORES, num_swdge_queues=NQ)

    xT = nc.dram_tensor("xT", [D, ns], BF16, kind="ExternalInput").ap()
    wts = nc.dram_tensor("wts", [D, 2 * D], BF16, kind="ExternalInput").ap()
    consts = nc.dram_tensor("consts", [D, 2 * D], BF16, kind="ExternalInput").ap()
    brow = nc.dram_tensor("brow", [1, 2 * D], BF16, kind="ExternalInput").ap()
    dinv_c = nc.dram_tensor("dinv_c", [D, nt], F32, kind="ExternalInput").ap()
    sdeg_r = nc.dram_tensor("sdeg_r", [1, nt * TILE], BF16, kind="ExternalInput").ap()
    idx_d = nc.dram_tensor("idx", [D, plan.idx_cols], I16, kind="ExternalInput").ap()
    idx0_d = nc.dram_tensor("idx0", [D, plan.idx0_cols], I16,
                            kind="ExternalInput").ap()
    doff_d = nc.dram_tensor("doff", [D, plan.ch_cols], F16, kind="ExternalInput").ap()
    iota16_d = nc.dram_tensor("iota16", [D, D], F16, kind="ExternalInput").ap()
    out_d = nc.dram_tensor("out", [ns, D], F32, kind="ExternalOutput").ap()

    hb = [nc.dram_tensor(f"h{i}b", [ns, D], BF16).ap() for i in range(2)]
    hf = [nc.dram_tensor(f"h{i}f", [n, D], BF16, addr_space="Shared").ap()
          for i in range(2)]

    max_C = max(int(plan.gb_C[g].sum()) for g in range(len(plan.groups)))
    max_icols = max_C * 8
    # one-hot columns per (g, b): sum over scheduled tiles of their k-span
    max_ohC = max(sum(nk for (_, _, nk) in plan.sched[g][b])
                  for g in range(len(plan.groups))
                  for b in range(plan.nblk))

    with tile.TileContext(nc) as tc:
        with (
            tc.tile_pool(name="const", bufs=1) as cpool,
            tc.tile_pool(name="xstream", bufs=3) as xpool,
            tc.tile_pool(name="stage", bufs=4) as spool,
            tc.tile_pool(name="pf", bufs=1) as pfpool,
            tc.tile_pool(name="oh", bufs=4) as ohpool,
            tc.tile_pool(name="aux", bufs=3) as auxpool,
            tc.tile_pool(name="ev", bufs=4) as evpool,
            tc.tile_pool(name="acc", bufs=5, space="PSUM") as accpool,
            tc.tile_pool(name="ptr", bufs=1, space="PSUM") as trpool,
            tc.tile_pool(name="pd", bufs=2, space="PSUM") as pdpool,
        ):
            w_sb = cpool.tile([D, 2 * D], BF16, tag="w")
            nc.sync.dma_start(out=w_sb[:], in_=wts[:])
            co_sb = cpool.tile([D, 2 * D], BF16, tag="co")
            nc.sync.dma_start(out=co_sb[:], in_=consts[:])
            br_sb = cpool.tile([1, 2 * D], BF16, tag="br")
            nc.sync.dma_start(out=br_sb[:], in_=brow[:])
            dv_sb = cpool.tile([D, nt], F32, tag="dv")
            nc.sync.dma_start(out=dv_sb[:], in_=dinv_c[:])
            sd_sb = cpool.tile([1, nt * TILE], BF16, tag="sd")
            nc.sync.dma_start(out=sd_sb[:], in_=sdeg_r[:])

            io_sb = cpool.tile([D, D], F16, tag="io16")
            nc.sync.dma_start(out=io_sb[:], in_=iota16_d[:])
            ix0_sb = cpool.tile([D, plan.idx0_cols], I16, tag="ix0")
            nc.sync.dma_start(out=ix0_sb[:], in_=idx0_d[:])

            W1 = w_sb[:, 0:D]
            W2 = w_sb[:, D:2 * D]
            iota = io_sb[:, 0:D]
            ident = co_sb[:, D:2 * D]

            # dense outputs h{0,1}' stay resident: the self-loop term is
            # added from here straight into PSUM (no gather round-trip)
            hself = [cpool.tile([TILE, nt * D], BF16, tag=f"hself{i}",
                                name=f"hself{i}")
                     for i in range(2)]

            def tw(t):
                return TILE if t < nt - 1 else plan.last_w

            # ---- layer-1 dense: h0' = (x @ W1) * dinv ----
            XB = 8  # xT tiles per DMA batch
            for t0 in range(0, nt, XB):
                tn = min(XB, nt - t0)
                wlast = tw(t0 + tn - 1)
                cols = (tn - 1) * TILE + wlast
                xt_t = xpool.tile([D, XB * TILE], BF16, tag="xt")
                nc.sync.dma_start(out=xt_t[:, :cols],
                                  in_=xT[:, t0 * TILE:t0 * TILE + cols])
                for t in range(t0, t0 + tn):
                    w = tw(t)
                    pd = pdpool.tile([TILE, D], F32, tag="pd")
                    nc.tensor.matmul(
                        pd[:w, :],
                        lhsT=xt_t[:, (t - t0) * TILE:(t - t0) * TILE + w],
                        rhs=W1, start=True, stop=True)
                    # dinv[row] is folded into xT on the host; plain copy,
                    # alternating engines to halve the eviction chain
                    hs = hself[0][:, t * D:(t + 1) * D]
                    if t % 2 == 0:
                        nc.scalar.activation(hs[:w, :], pd[:w, :], AF.Copy)
                    else:
                        nc.vector.tensor_copy(hs[:w, :], pd[:w, :])
                    nc.sync.dma_start(out=hb[0][t * TILE:t * TILE + w, :],
                                      in_=hs[:w, :])

            if stage == "dense":
                nc.gpsimd.collective_compute(
                    "AllGather", ALU.bypass,
                    replica_groups=[list(range(NCORES))],
                    ins=[hb[0].opt()], outs=[hf[0].opt()])
                for t in range(nt):
                    w = tw(t)
                    ev = evpool.tile([TILE, D], BF16, tag="ev")
                    nc.sync.dma_start(out=ev[:w, :],
                                      in_=hf[0][t * TILE:t * TILE + w, :])
                    evf = evpool.tile([TILE, D], F32, tag="evf")
                    nc.scalar.activation(evf[:w, :], ev[:w, :], AF.Copy)
                    nc.sync.dma_start(out=out_d[t * TILE:t * TILE + w, :],
                                      in_=evf[:w, :])

            # ---- sparse layer (templated over layer index) ----
            max_Cgb = max((int(plan.gb_C[g, b]) for g in range(len(plan.groups))
                           for b in range(nblk)), default=1)
            GMAX = 8  # dma_gather caps at ~1024 idxs (16KB desc ring)

            qcnt = [0]  # global dma_gather round-robin across SWDGE queues

            def sparse_layer(li):
                src_full = hf[li]
                # phase A: all groups' block-0 gathers (only need AG chunk 0)
                pftile = pfpool.tile([D, plan.pf_C, TILE], BF16, tag="pf")
                for c0 in range(0, plan.pf_C, GMAX):
                    cn = min(GMAX, plan.pf_C - c0)
                    nc.gpsimd.dma_gather(
                        pftile[:, c0:c0 + cn, :],
                        src_full[plan.blo[0]:plan.bhi[0], :],
                        ix0_sb[:, c0 * 8:(c0 + cn) * 8],
                        cn * TILE,
                        cn * TILE,
                        D,
                        queue_num=qcnt[0] % NQ,
                    )
                    qcnt[0] += 1
                # phase B: per-group one-hots + matmuls (b0 from the
                # prefetch tile) and b1.. gathers inline
                for g, tiles in enumerate(plan.groups):
                    Ctot = int(plan.gb_C[g].sum())
                    if Ctot == 0:
                        continue
                    pfed = g < plan.pf_ng   # b0 came from the prefetch pass
                    ibase = plan.idx_col[g][1] if pfed else plan.idx_col[g][0]
                    icols = plan.idx_col[g][0] + Ctot * 8 - ibase
                    idx_sb = auxpool.tile([D, max_icols], I16, tag="idx")
                    nc.sync.dma_start(
                        out=idx_sb[:, :icols],
                        in_=idx_d[:, ibase:ibase + icols])
                    do_sb = auxpool.tile([D, max_C], F16, tag="doff")
                    nc.sync.dma_start(
                        out=do_sb[:, :Ctot],
                        in_=doff_d[:, plan.ch_col[g]:plan.ch_col[g] + Ctot])

                    accs = {}
                    started = set()
                    for t in tiles:
                        accs[t] = accpool.tile([TILE, D], F32, tag="acc", name=f"acc_t{t}")
                        # self-loop term: acc[dst] += h'[dst]
                        w = tw(t)
                        nc.tensor.matmul(
                            accs[t][:w, :], lhsT=ident[:w, :w],
                            rhs=hself[li][:w, t * D:(t + 1) * D],
                            start=True, stop=False)
                        started.add(t)

                    gco = 0  # chunk offset within the group's doff columns
                    for b in range(nblk):
                        Cgb = int(plan.gb_C[g, b])
                        if Cgb == 0:
                            continue
                        sched = plan.sched[g][b]
                        ic0 = plan.idx_col[g][b] - ibase
                        if b == 0 and pfed:
                            stg = pftile[:, plan.pf_base[g]:
                                         plan.pf_base[g] + Cgb, :]
                        else:
                            stg = spool.tile([D, max_Cgb, TILE], BF16,
                                             tag="stage")
                        oh_sb = ohpool.tile([D, max_ohC, TILE], BF16, tag="oh")
                        # per-tile one-hots over each tile's chunk span,
                        # shifted so group offsets land on 0..127
                        oc = 0
                        ocol = {}
                        for (tr, kmin, nk) in sched:
                            ocol[tr] = oc
                            nc.vector.scalar_tensor_tensor(
                                out=oh_sb[:, oc:oc + nk, :],
                                in0=do_sb[:, gco + kmin:gco + kmin + nk]
                                    .unsqueeze(2).broadcast_to([D, nk, TILE]),
                                scalar=float(-TILE * tr),
                                in1=iota.unsqueeze(1).broadcast_to([D, nk, TILE]),
                                op0=ALU.add,
                                op1=ALU.is_equal,
                            )
                            oc += nk
                        if b > 0 or not pfed:
                            for c0 in range(0, Cgb, GMAX):
                                cn = min(GMAX, Cgb - c0)
                                nc.gpsimd.dma_gather(
                                    stg[:, c0:c0 + cn, :],
                                    src_full[plan.blo[b]:plan.bhi[b], :],
                                    idx_sb[:, ic0 + c0 * 8:ic0 + (c0 + cn) * 8],
                                    cn * TILE,
                                    cn * TILE,
                                    D,
                                    queue_num=qcnt[0] % NQ,
                                )
                                qcnt[0] += 1
                        for (tr, kmin, nk) in sched:
                            t = tiles[tr]
                            for j in range(nk):
                                nc.tensor.matmul(
                                    accs[t][:], lhsT=oh_sb[:, ocol[tr] + j, :],
                                    rhs=stg[:, kmin + j, :],
                                    start=(t not in started), stop=False)
                                started.add(t)
                        gco += Cgb

                    for t in tiles:
                        if t not in started:
                            continue
                        w = tw(t)
                        acc = accs[t]
                        # bias as rank-1: outer(sqrt(deg), b); sdeg rows
                        # beyond the tile width are zero-padded on the host
                        nc.tensor.matmul(
                            acc[:],
                            lhsT=sd_sb[:, t * TILE:(t + 1) * TILE],
                            rhs=br_sb[:, li * D:(li + 1) * D],
                            start=False, stop=True)
                        if li == 0 and stage == "l1":
                            ev = evpool.tile([TILE, D], F32, tag="ev")
                            nc.scalar.activation(ev[:w, :], acc[:w, :], AF.Copy,
                                                 scale=dv_sb[:w, t:t + 1])
                            nc.sync.dma_start(
                                out=out_d[t * TILE:t * TILE + w, :],
                                in_=ev[:w, :])
                        elif li == 0:
                            # fused layer-2 dense: h1' = (out1 @ W2) * dinv
                            ev = evpool.tile([TILE, D], BF16, tag="ev")
                            nc.scalar.activation(ev[:w, :], acc[:w, :], AF.Copy,
                                                 scale=dv_sb[:w, t:t + 1])
                            ptr = trpool.tile([D, TILE], BF16, tag="ptr")
                            nc.tensor.transpose(ptr[:, :w], ev[:w, :],
                                                ident[:w, :w])
                            trs = evpool.tile([D, TILE], BF16, tag="trs")
                            nc.vector.tensor_copy(trs[:, :w], ptr[:, :w])
                            pd = pdpool.tile([TILE, D], F32, tag="pd")
                            nc.tensor.matmul(pd[:w, :], lhsT=trs[:, :w], rhs=W2,
                                             start=True, stop=True)
                            hs1 = hself[1][:, t * D:(t + 1) * D]
                            nc.scalar.activation(hs1[:w, :], pd[:w, :], AF.Copy,
                                                 scale=dv_sb[:w, t:t + 1])
                            nc.sync.dma_start(
                                out=hb[1][t * TILE:t * TILE + w, :],
                                in_=hs1[:w, :])
                        else:
                            ev = evpool.tile([TILE, D], F32, tag="ev")
                            nc.scalar.activation(ev[:w, :], acc[:w, :], AF.Copy,
                                                 scale=dv_sb[:w, t:t + 1])
                            nc.sync.dma_start(
                                out=out_d[t * TILE:t * TILE + w, :],
                                in_=ev[:w, :])

            def allgather_split(li):
                # four tile-aligned chunks -> hf in chunk-major layout, one
                # chunk per gather src-block (indices are built against it)
                for jc in range(4):
                    lo = plan.ag_base[jc]
                    hi = lo + plan.ag_szs[jc]
                    nc.gpsimd.collective_compute(
                        "AllGather", ALU.bypass,
                        replica_groups=[list(range(NCORES))],
                        ins=[hb[li][lo:hi, :].opt()],
                        outs=[hf[li][plan.blo[jc]:plan.bhi[jc], :].opt()])

            if stage != "dense":
                allgather_split(0)
                sparse_layer(0)
            if stage == "l1d":
                for t in range(nt):
                    w = tw(t)
                    ev3 = evpool.tile([TILE, D], BF16, tag="ev3")
                    nc.sync.dma_start(out=ev3[:w, :],
                                      in_=hb[1][t * TILE:t * TILE + w, :])
                    ev3f = evpool.tile([TILE, D], F32, tag="ev3f")
                    nc.scalar.activation(ev3f[:w, :], ev3[:w, :], AF.Copy)
                    nc.sync.dma_start(out=out_d[t * TILE:t * TILE + w, :],
                                      in_=ev3f[:w, :])
            if stage == "full":
                allgather_split(1)
                sparse_layer(1)

    nc.compile()
    return nc


def _install_ntff_hook():
    """antenv.axon_hooks is absent in this image; synthesize it and register
    the ctypes NTFF profile hook from the boot module."""
    import types
    if "antenv.axon_hooks" in sys.modules:
        return
    try:
        from trn_agent_boot.trn_boot import _ntff_profile_via_ctypes
        hook = _ntff_profile_via_ctypes("/opt/axon/libaxon_pjrt.so")
    except Exception as e:
        print(f"[kernel] ntff hook unavailable: {e}", flush=True)
        hook = None
    mod = types.ModuleType("antenv.axon_hooks")
    mod._hook = hook
    mod.set_axon_ntff_profile_hook = lambda h: setattr(mod, "_hook", h)
    mod.get_axon_ntff_profile_hook = lambda: mod._hook
    sys.modules["antenv.axon_hooks"] = mod
    import antenv
    antenv.axon_hooks = mod


def _run(plan, x, W1, b1, W2, b2, trace=False, stage="full"):
    import time
    if trace:
        _install_ntff_hook()
    t0 = time.time()
    nc = _build(plan, stage=stage)
    t1 = time.time()
    if os.environ.get("GCN_VERBOSE"):
        print(f"[kernel] build+compile: {t1 - t0:.1f}s", flush=True)
    ns, nt = plan.ns, plan.nt
    iota_t = np.tile(np.arange(TILE, dtype=np.float32), (TILE, 1))
    ident_t = np.eye(TILE, dtype=np.float32)
    consts = np.concatenate([iota_t, ident_t], axis=1).astype(NPBF16)
    iota16 = iota_t.astype(np.float16)
    wts = np.concatenate([W1.astype(np.float32), W2.astype(np.float32)],
                         axis=1).astype(NPBF16)
    brow = np.concatenate([b1.astype(np.float32), b2.astype(np.float32)]
                          ).reshape(1, 2 * D).astype(NPBF16)

    in_maps = []
    for c in range(NCORES):
        lo, hi = c * ns, (c + 1) * ns
        dv = plan.dinv[lo:hi]
        # column t of dcol holds dinv[lo + t*128 : lo + (t+1)*128] (pad 1.0)
        dcol = np.ones((nt, TILE), dtype=np.float32)
        dcol.reshape(-1)[:ns] = dv
        dcol = np.ascontiguousarray(dcol.T)
        sdr = np.zeros((1, nt * TILE), dtype=np.float32)
        sdr[0, :ns] = plan.sdeg[lo:hi]
        idx, idx0, doff = plan.core_inputs(c)
        in_maps.append({
            "xT": np.ascontiguousarray(
                (x[lo:hi].astype(np.float32) * dv[:, None]).T).astype(NPBF16),
            "wts": wts, "consts": consts, "brow": brow, "iota16": iota16,
            "dinv_c": dcol, "sdeg_r": sdr.astype(NPBF16),
            "idx": idx, "idx0": idx0, "doff": doff,
        })
    t2 = time.time()
    res = run_bass_kernel_spmd(nc, in_maps, core_ids=list(range(NCORES)),
                               trace=trace)
    if os.environ.get("GCN_VERBOSE"):
        print(f"[kernel] prep inputs: {t2 - t1:.1f}s, run: {time.time() - t2:.1f}s",
              flush=True)
    out = np.concatenate([res.results[c]["out"] for c in range(NCORES)], axis=0)
    return out, res


def kernel(x, edge_index, W1, b1, W2, b2):
    plan = Plan(x.shape[0], np.asarray(edge_index))
    out, _ = _run(plan, np.asarray(x), np.asarray(W1), np.asarray(b1),
                  np.asarray(W2), np.asarray(b2))
    return out

